# revision 1
# baseline (speedup 1.0000x reference)
"""DTFBlock Trainium2 kernel: 8-core SPMD (batch x H-half sharding).

Per-core layout: partition = (t*8 + c_local) over 8-channel chunks, free =
padded spatial frame [30, 60] (interior 28x56 at row 1 / col 2).  All
contractions run on PE via host-built block-diagonal / selection matrices;
3x3 shifts are free-axis AP offsets; the frame shift (t+1) is a partition
offset; BN statistics are AllReduced across the 8 cores.
"""

import numpy as np
import ml_dtypes

import bass_rust
import concourse.bass as bass
import concourse.tile as tile
from concourse import mybir
from concourse.bass_utils import run_bass_kernel_spmd

# --------------------------------------------------------------- patch ----
# This container's walrus rejects instructions carrying more than one sync
# wait: hoist extras into same-engine NOPs placed before the instruction.
_orig_sched = tile.TileContext.schedule_and_allocate


def _split_sync_waits(nc, max_waits=1):
    for f in nc.m.functions:
        for bb in f.blocks:
            il = list(bb.instructions)
            new = []
            changed = False
            for ins in il:
                si = ins.sync_info
                if si is not None and len(si.on_wait) > max_waits:
                    waits = list(si.on_wait)
                    for j, w in enumerate(waits[:-max_waits]):
                        nop = bass_rust.InstNoOp(
                            name=f"{ins.name}-ws{j}",
                            engine=ins.engine,
                            ins=[],
                            outs=[],
                            sync_info=bass_rust.SyncInfo(on_wait=[w], on_update=[]),
                        )
                        new.append(nop)
                        changed = True
                    ins.sync_info = bass_rust.SyncInfo(
                        on_wait=waits[-max_waits:], on_update=list(si.on_update)
                    )
                new.append(ins)
            if changed:
                bb.instructions = new


def _patched_sched(self, *a, **k):
    r = _orig_sched(self, *a, **k)
    _split_sync_waits(self.nc)
    return r


if tile.TileContext.schedule_and_allocate.__name__ != "_patched_sched":
    tile.TileContext.schedule_and_allocate = _patched_sched

# --------------------------------------------------------------- consts ---
B, C, T, H, W = 4, 128, 16, 56, 56
K = 3
NF = T // 2 + 1          # 9 rfft bins
KK = K * K
EPS = 1e-5
Hh = H // 2              # 28 rows per core
HP, WP = Hh + 2, W + 4   # padded frame 30 x 60
FR = HP * WP             # 1800
HWI = Hh * W             # 1568
CL = 8                   # channels per chunk
NCC = C // CL            # 16 chunks
NQ = 2 * NF              # 18 (ri, f); q = ri*9 + f
SOFF = 64                # s-frame margin
SFR = 2048               # s-frame row length
NBN = B * NF * H * W     # BN count per channel
F32 = mybir.dt.float32
BF16 = mybir.dt.bfloat16
BF = ml_dtypes.bfloat16
NCHUNK, CHH, CHW = 4, 7, 392  # hw chunks: 4 x (7 rows * 56)

DELTAS = [(ki - 1, kj - 1) for ki in range(K) for kj in range(K)]


def _dft_mats():
    Fm = np.fft.rfft(np.eye(T), axis=0, norm="ortho")  # [9, 16]
    Mr = np.zeros((T, NF))
    Mi = np.zeros((T, NF))
    for f in range(NF):
        e = np.zeros(NF, complex)
        e[f] = 1.0
        Mr[:, f] = np.fft.irfft(e, n=T, norm="ortho")
        Mi[:, f] = np.fft.irfft(1j * e, n=T, norm="ortho")
    return Fm.real.copy(), Fm.imag.copy(), Mr, Mi


def _build_consts(weights_cor, Wg, w1r, w1i, Wlr, Wli,
                  gamma_r, beta_r, gamma_i, beta_i, alpha1, bias_p):
    c = {}
    f4 = np.float32
    wc = np.zeros((128, KK, NCC, T), f4)
    for kk in range(KK):
        ki, kj = kk // K, kk % K
        for cc in range(NCC):
            for cl in range(CL):
                for t in range(T):
                    wc[t * CL + cl, kk, cc, t] = weights_cor[cc * CL + cl, t, ki, kj]
    c["wc"] = wc.astype(BF)
    # aggregation identity with the frame shift folded in:
    # out[m] = tau_sum[m + 8]  (t -> t+1 shift; rows 120..127 become 0)
    ish = np.zeros((128, 128), f4)
    for m in range(120):
        ish[m + 8, m] = 1.0
    c["ident_shift"] = ish.astype(BF)
    rep16 = np.zeros((T, 128), f4)
    for t in range(T):
        rep16[t, t * CL:(t + 1) * CL] = 1.0
    c["rep16"] = rep16.astype(BF)
    c["eye16"] = np.eye(T, dtype=f4).astype(BF)

    Fr, Fi, Mr, Mi = _dft_mats()
    rfr = np.zeros((128, CL * NF), f4)
    rfi = np.zeros((128, CL * NF), f4)
    for t in range(T):
        for cl in range(CL):
            for f in range(NF):
                rfr[t * CL + cl, cl * NF + f] = Fr[f, t]
                rfi[t * CL + cl, cl * NF + f] = Fi[f, t]
    c["rfftR"] = rfr.astype(BF)
    c["rfftI"] = rfi.astype(BF)

    wg_a = np.zeros((128, NCC, 128), f4)
    wg_b = np.zeros((128, NCC, 16), f4)
    for cc in range(NCC):
        for t in range(T):
            for cl in range(CL):
                col = (cc * CL + cl) * T + t
                wg_a[t * CL + cl, cc, :] = Wg[:128, col]
                wg_b[t * CL + cl, cc, :] = Wg[128:, col]
    c["wg_a"] = wg_a.astype(BF)
    c["wg_b"] = wg_b.astype(BF)
    wo_a = np.zeros((128, 128), f4)
    wo_b = np.zeros((128, 16), f4)
    wo2_a = np.zeros((16, 128), f4)
    wo2_b = np.zeros((16, 16), f4)
    for t in range(T):
        for kk in range(8):
            col = (C + kk) * T + t
            wo_a[t * 8 + kk, :] = Wg[:128, col]
            wo_b[t * 8 + kk, :] = Wg[128:, col]
        col = (C + 8) * T + t
        wo2_a[t, :] = Wg[:128, col]
        wo2_b[t, :] = Wg[128:, col]
    c["wg_wo_a"] = wo_a.astype(BF)
    c["wg_wo_b"] = wo_b.astype(BF)
    c["wg_wo2_a"] = wo2_a.astype(BF)
    c["wg_wo2_b"] = wo2_b.astype(BF)

    w1A = np.zeros((NCC, 144, CL * NF), f4)
    w1B = np.zeros((NCC, 144, CL * NF), f4)
    for cc in range(NCC):
        for cl in range(CL):
            ch = cc * CL + cl
            for f in range(NF):
                o_r = (18 * ch + 2 * f) // 16
                o_i = (18 * ch + 2 * f + 1) // 16
                m = cl * NF + f
                w1A[cc, o_r, m] += w1r[ch, f]
                w1A[cc, o_i, m] -= w1i[ch, f]
                w1B[cc, o_r, m] += w1i[ch, f]
                w1B[cc, o_i, m] += w1r[ch, f]
    c["w1A"] = w1A
    c["w1B"] = w1B
    c["wlrT"] = np.ascontiguousarray(Wlr.T).astype(BF)
    c["wliT"] = np.ascontiguousarray(Wli.T).astype(BF)
    c["wliTn"] = np.ascontiguousarray(-Wli.T).astype(BF)

    # irfft block-diag over z-chunk rows k = cl*18 + q; K split 126 + 18
    Mhat = np.concatenate([Mr, Mi], axis=1)  # [16, 18]
    ir_a = np.zeros((126, 128), f4)
    ir_b = np.zeros((18, 128), f4)
    for cl in range(7):
        for q in range(NQ):
            ir_a[cl * NQ + q, cl * T:(cl + 1) * T] = Mhat[:, q]
    for q in range(NQ):
        ir_b[q, 7 * T:8 * T] = Mhat[:, q]
    c["irfft_a"] = ir_a.astype(BF)
    c["irfft_b"] = ir_b.astype(BF)

    # K1 / D selection matmuls: out[m] = sum_o sel[o, m] * vec[o]
    k1sel_r = np.zeros((NCC, 128, 128), f4)
    k1sel_i = np.zeros((NCC, 128, 128), f4)
    k1selb_r = np.zeros((NCC, 128, 18), f4)
    k1selb_i = np.zeros((NCC, 128, 18), f4)
    dsel_r = np.zeros((NCC, 128, 128), f4)
    dsel_i = np.zeros((NCC, 128, 128), f4)
    dsel_bias = np.zeros((NCC, 128, 128), f4)
    SMr = Mr.sum(axis=1)
    SMi = Mi.sum(axis=1)
    for cc in range(NCC):
        for cl in range(7):
            o = cc * CL + cl
            for q in range(NQ):
                m = cl * NQ + q
                (k1sel_r if q < NF else k1sel_i)[cc, o, m] = 1.0
        o = cc * CL + 7
        for q in range(NQ):
            (k1selb_r if q < NF else k1selb_i)[cc, o, q] = 1.0
        for cl in range(CL):
            o = cc * CL + cl
            for t in range(T):
                m = cl * T + t
                dsel_r[cc, o, m] = SMr[t]
                dsel_i[cc, o, m] = SMi[t]
                dsel_bias[cc, o, m] = 1.0
    c["k1sel_r"] = k1sel_r
    c["k1sel_i"] = k1sel_i
    c["k1selb_r"] = k1selb_r
    c["k1selb_i"] = k1selb_i
    c["dsel_r"] = dsel_r
    c["dsel_i"] = dsel_i
    c["dsel_bias"] = dsel_bias

    aff = np.zeros((128, 8), f4)
    aff[:, 0] = gamma_r
    aff[:, 1] = gamma_i
    aff[:, 2] = beta_r
    aff[:, 3] = beta_i
    aff[:, 4] = alpha1.reshape(-1)
    aff[:, 5] = bias_p.reshape(-1)
    c["aff"] = aff
    return c


# ---------------------------------------------------------- bass program --
_CACHE = {}

AO = mybir.AluOpType
AX = mybir.AxisListType
AF = mybir.ActivationFunctionType


def _build_program(repeat=1):
    nc = bass.Bass("TRN2", target_bir_lowering=False, debug=False, num_devices=8)

    def din(name, shape, dtype=F32):
        return nc.dram_tensor(name, list(shape), dtype, kind="ExternalInput").ap()

    x_bf_d = din("x_bf", [NCC, 128, HP, WP], BF16)
    xn_c_d = din("xn_c", [NCC, 128, HP, WP], BF16)
    xn_m_d = din("xn_m", [NCC, 128, HP, WP], BF16)
    xn_p_d = din("xn_p", [NCC, 128, HP, WP], BF16)
    x_id_d = din("x_id", [NCC, 128, Hh, W], F32)
    wc_d = din("wc", [128, KK, NCC, T], BF16)
    ish_d = din("ident_shift", [128, 128], BF16)
    rep16_d = din("rep16", [T, 128], BF16)
    eye16_d = din("eye16", [T, T], BF16)
    rfr_d = din("rfftR", [128, CL * NF], BF16)
    rfi_d = din("rfftI", [128, CL * NF], BF16)
    wga_d = din("wg_a", [128, NCC, 128], BF16)
    wgb_d = din("wg_b", [128, NCC, 16], BF16)
    wgwoa_d = din("wg_wo_a", [128, 128], BF16)
    wgwob_d = din("wg_wo_b", [128, 16], BF16)
    wgwo2a_d = din("wg_wo2_a", [16, 128], BF16)
    wgwo2b_d = din("wg_wo2_b", [16, 16], BF16)
    w1A_d = din("w1A", [NCC, 144, CL * NF], F32)
    w1B_d = din("w1B", [NCC, 144, CL * NF], F32)
    wlrT_d = din("wlrT", [128, 128], BF16)
    wliT_d = din("wliT", [128, 128], BF16)
    wliTn_d = din("wliTn", [128, 128], BF16)
    ira_d = din("irfft_a", [126, 128], BF16)
    irb_d = din("irfft_b", [18, 128], BF16)
    k1r_d = din("k1sel_r", [NCC, 128, 128], F32)
    k1i_d = din("k1sel_i", [NCC, 128, 128], F32)
    k1br_d = din("k1selb_r", [NCC, 128, 18], F32)
    k1bi_d = din("k1selb_i", [NCC, 128, 18], F32)
    dsr_d = din("dsel_r", [NCC, 128, 128], F32)
    dsi_d = din("dsel_i", [NCC, 128, 128], F32)
    dsb_d = din("dsel_bias", [NCC, 128, 128], F32)
    aff_d = din("aff", [128, 8], F32)

    out_d = nc.dram_tensor("out", [NCC, 128, Hh, W], F32,
                           kind="ExternalOutput").ap()
    y_dram = nc.dram_tensor("y_stage", [2, NCC, CL, NF, HWI], BF16,
                            kind="Internal").ap()
    z_dram = nc.dram_tensor("z_stage", [C, NQ, HWI], BF16, kind="Internal").ap()
    wo_dram = nc.dram_tensor("wo_stage", [16, 8, HWI], F32, kind="Internal").ap()
    wo2_dram = nc.dram_tensor("wo2_stage", [16, HWI], F32, kind="Internal").ap()

    from contextlib import ExitStack

    with tile.TileContext(nc) as tc, ExitStack() as ctx:
        cst = ctx.enter_context(tc.tile_pool(name="cst", bufs=1))
        sfp = ctx.enter_context(tc.tile_pool(name="sfp", bufs=1))
        dram_p = ctx.enter_context(tc.tile_pool(name="dramp", bufs=2, space="DRAM"))

        # ---- consts to SBUF
        wc_sb = cst.tile([128, KK, NCC, T], BF16, tag="wc")
        nc.sync.dma_start(out=wc_sb, in_=wc_d)
        ish = cst.tile([128, 128], BF16, tag="ish")
        nc.sync.dma_start(out=ish, in_=ish_d)
        rep16 = cst.tile([T, 128], BF16, tag="rep16")
        nc.sync.dma_start(out=rep16, in_=rep16_d)
        eye16 = cst.tile([T, T], BF16, tag="eye16")
        nc.sync.dma_start(out=eye16, in_=eye16_d)
        rfr = cst.tile([128, CL * NF], BF16, tag="rfr")
        nc.sync.dma_start(out=rfr, in_=rfr_d)
        rfi = cst.tile([128, CL * NF], BF16, tag="rfi")
        nc.sync.dma_start(out=rfi, in_=rfi_d)
        wga = cst.tile([128, NCC, 128], BF16, tag="wga")
        nc.sync.dma_start(out=wga, in_=wga_d)
        wgb = cst.tile([128, NCC, 16], BF16, tag="wgb")
        nc.sync.dma_start(out=wgb, in_=wgb_d)
        wgwoa = cst.tile([128, 128], BF16, tag="wgwoa")
        nc.sync.dma_start(out=wgwoa, in_=wgwoa_d)
        wgwob = cst.tile([128, 16], BF16, tag="wgwob")
        nc.sync.dma_start(out=wgwob, in_=wgwob_d)
        wgwo2a = cst.tile([16, 128], BF16, tag="wgwo2a")
        nc.sync.dma_start(out=wgwo2a, in_=wgwo2a_d)
        wgwo2b = cst.tile([16, 16], BF16, tag="wgwo2b")
        nc.sync.dma_start(out=wgwo2b, in_=wgwo2b_d)
        wlrT = cst.tile([128, 128], BF16, tag="wlrT")
        nc.sync.dma_start(out=wlrT, in_=wlrT_d)
        wliT = cst.tile([128, 128], BF16, tag="wliT")
        nc.sync.dma_start(out=wliT, in_=wliT_d)
        wliTn = cst.tile([128, 128], BF16, tag="wliTn")
        nc.sync.dma_start(out=wliTn, in_=wliTn_d)
        ira = cst.tile([126, 128], BF16, tag="ira")
        nc.sync.dma_start(out=ira, in_=ira_d)
        irb = cst.tile([18, 128], BF16, tag="irb")
        nc.sync.dma_start(out=irb, in_=irb_d)
        aff = cst.tile([128, 8], F32, tag="aff")
        nc.sync.dma_start(out=aff, in_=aff_d)

        # ---- persistent small working tensors
        acc_s = sfp.tile([128, NQ], F32, tag="acc_s")
        acc_q = sfp.tile([128, NQ], F32, tag="acc_q")
        stats = sfp.tile([128, 4], F32, tag="stats")
        rstats = sfp.tile([128, 4], F32, tag="rstats")
        kw = sfp.tile([128, 12], F32, tag="kw")
        tmpk = sfp.tile([128, 4], F32, tag="tmpk")
        epst = sfp.tile([128, 1], F32, tag="epst")
        nc.vector.memset(epst, EPS)

        for _rep in range(repeat):
            # =========== Phases 1 + 2 ==========================================
            # s-frames: one [16, SFR] bf16 frame per kk (exp(w_o) written into
            # the interior, then scaled in place by 1/sum to become the softmax
            # weights, zero margin elsewhere).
            x2_pool = tc.tile_pool(name="x2p", bufs=1)
            x2p = x2_pool.__enter__()
            srep_pool = tc.tile_pool(name="srep", bufs=1)
            srp = srep_pool.__enter__()
            sf_pool = tc.tile_pool(name="sfpool", bufs=1)
            sfE = sf_pool.__enter__()
            sframes = []
            for kk in range(KK):
                sf = sfE.tile([16, SFR], BF16, tag=f"sf{kk}", name=f"sf{kk}")
                nc.gpsimd.memset(sf, 0.0)
                sframes.append(sf)
            rD16 = sfE.tile([16, NCHUNK, CHW], F32, tag="rD16")

            def sf_int(kk, j=None):
                v = sframes[kk][:, SOFF:SOFF + FR].rearrange(
                    "p (h w) -> p h w", h=HP)
                if j is None:
                    return v[:, 1:1 + Hh, 2:2 + W]
                return v[:, 1 + CHH * j:1 + CHH * (j + 1), 2:2 + W]

            with tc.tile_pool(name="wops", bufs=1, space="PSUM") as wop, \
                    tc.tile_pool(name="xst", bufs=3) as xst, \
                    tc.tile_pool(name="xnp", bufs=2) as xnp, \
                    tc.tile_pool(name="vp", bufs=2) as v_p, \
                    tc.tile_pool(name="wotp", bufs=2) as wotp:
                wo_ps = [wop.tile([128, 512], F32, tag=f"wops{kk}", name=f"wops{kk}")
                         for kk in range(8)]

                def corr_mm(pst, kk, cc, v, first, last):
                    for j in range(NCHUNK):
                        nc.tensor.matmul(
                            pst[32 * j:32 * j + 16, 0:CHW],
                            lhsT=wc_sb[:, kk, cc, :],
                            rhs=v[:, CHH * j:CHH * (j + 1), :],
                            start=first,
                            stop=last,
                            tile_position=(0, 32 * j),
                        )

                def evict_wo(pst, kk, dram_ap):
                    wt = wotp.tile([16, Hh, W], F32, tag="wotmp", name=f"wot{kk}")
                    for j in range(NCHUNK):
                        pin = pst[32 * j:32 * j + 16, 0:CHW].rearrange(
                            "p (hh w) -> p hh w", hh=CHH)
                        nc.scalar.activation(
                            out=wt[:, CHH * j:CHH * (j + 1), :],
                            in_=pin, func=AF.Copy)
                        nc.scalar.activation(
                            out=sf_int(kk, j), in_=pin, func=AF.Exp)
                    nc.sync.dma_start(
                        out=dram_ap, in_=wt.rearrange("p h w -> p (h w)"))

                for cc in range(NCC):
                    xt = xst.tile([128, HP, WP], BF16, tag="xs", name=f"xs{cc}")
                    nc.sync.dma_start(out=xt, in_=x_bf_d[cc])
                    var_tiles = {}
                    for dw, d in ((0, xn_c_d), (-1, xn_m_d), (1, xn_p_d)):
                        vt = xnp.tile([128, HP, WP], BF16, tag=f"xn{dw}",
                                      name=f"xn{dw}_{cc}")
                        nc.sync.dma_start(out=vt, in_=d[cc])
                        var_tiles[dw] = vt
                    for kk in range(8):
                        dh, dw = DELTAS[kk]
                        v = v_p.tile([128, Hh, W], BF16, tag="v", name=f"v{cc}_{kk}")
                        nc.vector.tensor_tensor(
                            out=v,
                            in0=xt[:, 1:1 + Hh, 2:2 + W],
                            in1=var_tiles[dw][:, 1 + dh:1 + dh + Hh, 2:2 + W],
                            op=AO.mult,
                        )
                        corr_mm(wo_ps[kk], kk, cc, v, cc == 0, cc == NCC - 1)
                for kk in range(8):
                    evict_wo(wo_ps[kk], kk, wo_dram[:, kk, :])
                # kk8 = (1, 1): second streaming pass, reusing the wops0 bank
                p8 = wop.tile([128, 512], F32, tag="wops0", name="wops8")
                for cc in range(NCC):
                    xt = xst.tile([128, HP, WP], BF16, tag="xs", name=f"xs8_{cc}")
                    nc.sync.dma_start(out=xt, in_=x_bf_d[cc])
                    vt = xnp.tile([128, HP, WP], BF16, tag="xn1", name=f"xn8_{cc}")
                    nc.sync.dma_start(out=vt, in_=xn_p_d[cc])
                    v = v_p.tile([128, Hh, W], BF16, tag="v", name=f"v8_{cc}")
                    nc.vector.tensor_tensor(
                        out=v,
                        in0=xt[:, 1:1 + Hh, 2:2 + W],
                        in1=vt[:, 2:2 + Hh, 2:2 + W],
                        op=AO.mult,
                    )
                    corr_mm(p8, 8, cc, v, cc == 0, cc == NCC - 1)
                evict_wo(p8, 8, wo2_dram)

            # ---- softmax: denominators + normalize the frames in place
            ps_pool = tc.tile_pool(name="ps", bufs=2, space="PSUM")
            ps = ps_pool.__enter__()
            pd = ps.tile([128, NCHUNK, 512], F32, tag="ps", name="pdsum")
            for j in range(NCHUNK):
                for kk in range(KK):
                    nc.tensor.matmul(
                        pd[0:16, j, 0:CHW], lhsT=eye16,
                        rhs=sf_int(kk, j), start=(kk == 0), stop=(kk == KK - 1))
            with nc.allow_low_precision(reason="softmax denominators ~9, bf16 ok"):
                nc.vector.reciprocal(out=rD16, in_=pd[0:16, :, 0:CHW])
            for kk in range(KK):
                nc.vector.tensor_tensor(
                    out=sf_int(kk),
                    in0=sf_int(kk),
                    in1=rD16.rearrange("p c (hh w) -> p (c hh) w", hh=CHH),
                    op=AO.mult)

            # =========== Phase 1c: s' replication ==============================
            NS = 450
            srep = []
            for kk in range(KK):
                dh, dw = DELTAS[kk]
                delta = dh * WP + dw
                p = ps.tile([128, NCHUNK, 512], F32, tag="ps", name=f"psrep{kk}")
                for m in range(NCHUNK):
                    nc.tensor.matmul(
                        p[:, m, 0:NS], lhsT=rep16,
                        rhs=sframes[kk][:, SOFF - delta + NS * m:
                                        SOFF - delta + NS * (m + 1)],
                        start=True, stop=True)
                st = srp.tile([128, FR], BF16, tag=f"srep{kk}", name=f"srep{kk}")
                nc.scalar.activation(
                    out=st.rearrange("p (c x) -> p c x", c=NCHUNK),
                    in_=p[:, :, 0:NS], func=AF.Copy)
                srep.append(st)
            sf_pool.__exit__(None, None, None)

            # =========== Phase 2: aggregation + x2 =============================
            x2 = []
            with tc.tile_pool(name="xst2", bufs=3) as xst2, \
                    tc.tile_pool(name="taup", bufs=3) as tau_p:
                for cc in range(NCC):
                    xt = xst2.tile([128, HP, WP], BF16, tag="xs2", name=f"xs2_{cc}")
                    nc.sync.dma_start(out=xt, in_=x_bf_d[cc])
                    agg = ps.tile([128, NCHUNK, 512], F32, tag="ps", name=f"agg{cc}")
                    for kk in range(KK):
                        dh, dw = DELTAS[kk]
                        tau = tau_p.tile([128, HP, WP], BF16, tag="tau",
                                         name=f"tau{cc}_{kk}")
                        nc.vector.tensor_tensor(
                            out=tau.rearrange("p h w -> p (h w)"),
                            in0=srep[kk],
                            in1=xt.rearrange("p h w -> p (h w)"),
                            op=AO.mult)
                        for j in range(NCHUNK):
                            nc.tensor.matmul(
                                agg[:, j, 0:CHW],
                                lhsT=ish,
                                rhs=tau[:, 1 + CHH * j + dh:1 + CHH * (j + 1) + dh,
                                        2 + dw:2 + dw + W],
                                start=(kk == 0),
                                stop=(kk == KK - 1))
                    xv = x2p.tile([128, NCHUNK, CHW], BF16, tag=f"x2{cc}",
                                  name=f"x2{cc}")
                    # x2 = a_shift + x  (shift already folded into ish)
                    nc.vector.tensor_tensor(
                        out=xv.rearrange("p c (hh w) -> p c hh w", hh=CHH),
                        in0=agg[:, :, 0:CHW].rearrange(
                            "p c (hh w) -> p c hh w", hh=CHH),
                        in1=xt[:, 1:1 + Hh, 2:2 + W].rearrange(
                            "p (c hh) w -> p c hh w", hh=CHH),
                        op=AO.add)
                    x2.append(xv)
            srep_pool.__exit__(None, None, None)

            # w_o tiles for the Wg input (loaded from the DRAM staging copies)
            itp_pool = tc.tile_pool(name="itp", bufs=1)
            itp = itp_pool.__enter__()
            wo_t1 = itp.tile([128, NCHUNK, CHW], BF16, tag="wo_t1")
            wo_t2 = itp.tile([16, NCHUNK, CHW], BF16, tag="wo_t2")
            it_a = itp.tile([128, NCHUNK, CHW], F32, tag="it_a")
            it_b = itp.tile([16, NCHUNK, CHW], F32, tag="it_b")
            nc.gpsimd.dma_start(
                out=wo_t1.rearrange("p c w -> p (c w)"),
                in_=wo_dram.rearrange("t k x -> (t k) x"))
            nc.gpsimd.dma_start(
                out=wo_t2.rearrange("p c w -> p (c w)"),
                in_=wo2_dram)

            # =========== Phase B1: Wg 1x1 conv -> iterm ========================
            pg1 = ps.tile([128, NCHUNK, 512], F32, tag="ps", name="pg1")
            pg2 = ps.tile([128, NCHUNK, 512], F32, tag="ps", name="pg2")
            for j in range(NCHUNK):
                for cc in range(NCC):
                    nc.tensor.matmul(pg1[:, j, 0:CHW], lhsT=wga[:, cc, :],
                                     rhs=x2[cc][:, j, :], start=(cc == 0), stop=False)
                nc.tensor.matmul(pg1[:, j, 0:CHW], lhsT=wgwoa,
                                 rhs=wo_t1[:, j, :], start=False, stop=False)
                nc.tensor.matmul(pg1[:, j, 0:CHW], lhsT=wgwo2a,
                                 rhs=wo_t2[:, j, :], start=False, stop=True)
            nc.scalar.activation(out=it_a, in_=pg1[:, :, 0:CHW], func=AF.Copy)
            for j in range(NCHUNK):
                for cc in range(NCC):
                    nc.tensor.matmul(pg2[0:16, j, 0:CHW], lhsT=wgb[:, cc, :],
                                     rhs=x2[cc][:, j, :], start=(cc == 0), stop=False)
                nc.tensor.matmul(pg2[0:16, j, 0:CHW], lhsT=wgwob,
                                 rhs=wo_t1[:, j, :], start=False, stop=False)
                nc.tensor.matmul(pg2[0:16, j, 0:CHW], lhsT=wgwo2b,
                                 rhs=wo_t2[:, j, :], start=False, stop=True)
            nc.scalar.activation(out=it_b, in_=pg2[0:16, :, 0:CHW], func=AF.Copy)

            # =========== Phase B2: rfft + dynamic filter + pointwise ==========
            with tc.tile_pool(name="pwp", bufs=2) as pw_p, \
                    tc.tile_pool(name="w1p", bufs=2) as w1p:
                for cc in range(NCC):
                    pr = ps.tile([128, NCHUNK, 512], F32, tag="ps", name=f"prf{cc}")
                    pi = ps.tile([128, NCHUNK, 512], F32, tag="ps", name=f"pif{cc}")
                    for j in range(NCHUNK):
                        nc.tensor.matmul(pr[0:CL * NF, j, 0:CHW], lhsT=rfr,
                                         rhs=x2[cc][:, j, :], start=True, stop=True)
                        nc.tensor.matmul(pi[0:CL * NF, j, 0:CHW], lhsT=rfi,
                                         rhs=x2[cc][:, j, :], start=True, stop=True)
                    xtr = pw_p.tile([CL * NF, NCHUNK, CHW], BF16, tag="xtr",
                                    name=f"xtr{cc}")
                    xti = pw_p.tile([CL * NF, NCHUNK, CHW], BF16, tag="xti",
                                    name=f"xti{cc}")
                    nc.scalar.activation(out=xtr, in_=pr[0:CL * NF, :, 0:CHW],
                                         func=AF.Copy)
                    nc.scalar.activation(out=xti, in_=pi[0:CL * NF, :, 0:CHW],
                                         func=AF.Copy)
                    w1a1 = w1p.tile([128, CL * NF], F32, tag="w1a1", name=f"wa1{cc}")
                    nc.sync.dma_start(out=w1a1, in_=w1A_d[cc, 0:128, :])
                    w1a2 = w1p.tile([16, CL * NF], F32, tag="w1a2", name=f"wa2{cc}")
                    nc.sync.dma_start(out=w1a2, in_=w1A_d[cc, 128:144, :])
                    w1b1 = w1p.tile([128, CL * NF], F32, tag="w1b1", name=f"wb1{cc}")
                    nc.sync.dma_start(out=w1b1, in_=w1B_d[cc, 0:128, :])
                    w1b2 = w1p.tile([16, CL * NF], F32, tag="w1b2", name=f"wb2{cc}")
                    nc.sync.dma_start(out=w1b2, in_=w1B_d[cc, 128:144, :])
                    pa = ps.tile([128, NCHUNK, 512], F32, tag="ps", name=f"pa{cc}")
                    pb = ps.tile([128, NCHUNK, 512], F32, tag="ps", name=f"pb{cc}")
                    for j in range(NCHUNK):
                        nc.tensor.matmul(pa[0:CL * NF, j, 0:CHW], lhsT=w1a1,
                                         rhs=it_a[:, j, :], start=True, stop=False)
                        nc.tensor.matmul(pa[0:CL * NF, j, 0:CHW], lhsT=w1a2,
                                         rhs=it_b[:, j, :], start=False, stop=True)
                        nc.tensor.matmul(pb[0:CL * NF, j, 0:CHW], lhsT=w1b1,
                                         rhs=it_a[:, j, :], start=True, stop=False)
                        nc.tensor.matmul(pb[0:CL * NF, j, 0:CHW], lhsT=w1b2,
                                         rhs=it_b[:, j, :], start=False, stop=True)
                    Am = pw_p.tile([CL * NF, NCHUNK, CHW], BF16, tag="Am",
                                   name=f"Am{cc}")
                    Bm = pw_p.tile([CL * NF, NCHUNK, CHW], BF16, tag="Bm",
                                   name=f"Bm{cc}")
                    nc.scalar.activation(out=Am, in_=pa[0:CL * NF, :, 0:CHW],
                                         func=AF.Copy)
                    nc.scalar.activation(out=Bm, in_=pb[0:CL * NF, :, 0:CHW],
                                         func=AF.Copy)
                    t1 = pw_p.tile([CL * NF, NCHUNK, CHW], BF16, tag="t1",
                                   name=f"t1{cc}")
                    t2 = pw_p.tile([CL * NF, NCHUNK, CHW], BF16, tag="t2",
                                   name=f"t2{cc}")
                    yr = pw_p.tile([CL * NF, NCHUNK, CHW], BF16, tag="yr",
                                   name=f"yrt{cc}")
                    yi = pw_p.tile([CL * NF, NCHUNK, CHW], BF16, tag="yi",
                                   name=f"yit{cc}")
                    nc.vector.tensor_tensor(out=t1, in0=xtr, in1=Am, op=AO.mult)
                    nc.vector.tensor_tensor(out=t2, in0=xti, in1=Bm, op=AO.mult)
                    nc.vector.tensor_tensor(out=yr, in0=t1, in1=t2, op=AO.subtract)
                    nc.vector.tensor_tensor(out=t1, in0=xtr, in1=Bm, op=AO.mult)
                    nc.vector.tensor_tensor(out=t2, in0=xti, in1=Am, op=AO.mult)
                    nc.vector.tensor_tensor(out=yi, in0=t1, in1=t2, op=AO.add)
                    nc.sync.dma_start(
                        out=y_dram[0, cc].rearrange("c f x -> (c f) x"),
                        in_=yr.rearrange("p c x -> p (c x)"))
                    nc.sync.dma_start(
                        out=y_dram[1, cc].rearrange("c f x -> (c f) x"),
                        in_=yi.rearrange("p c x -> p (c x)"))
            itp_pool.__exit__(None, None, None)
            x2_pool.__exit__(None, None, None)

            # =========== Phase B3: FFTLinear + BN stats ========================
            with tc.tile_pool(name="yld", bufs=3) as yld, \
                    tc.tile_pool(name="zev", bufs=3) as zev:
                sq_scr = zev.tile([128, HWI], BF16, tag="sq_scr", bufs=1)
                for f in range(NF):
                    yrf = yld.tile([128, NCHUNK, CHW], BF16, tag="yrf",
                                   name=f"yrf{f}")
                    nc.sync.dma_start(
                        out=yrf.rearrange("p c x -> p (c x)"),
                        in_=y_dram[0, :, :, f, :].rearrange("n c x -> (n c) x"))
                    yif = yld.tile([128, NCHUNK, CHW], BF16, tag="yif",
                                   name=f"yif{f}")
                    nc.sync.dma_start(
                        out=yif.rearrange("p c x -> p (c x)"),
                        in_=y_dram[1, :, :, f, :].rearrange("n c x -> (n c) x"))
                    pr = ps.tile([128, NCHUNK, 512], F32, tag="ps", name=f"pfr{f}")
                    pi = ps.tile([128, NCHUNK, 512], F32, tag="ps", name=f"pfi{f}")
                    for j in range(NCHUNK):
                        nc.tensor.matmul(pr[:, j, 0:CHW], lhsT=wlrT,
                                         rhs=yrf[:, j, :], start=True, stop=False)
                        nc.tensor.matmul(pr[:, j, 0:CHW], lhsT=wliTn,
                                         rhs=yif[:, j, :], start=False, stop=True)
                        nc.tensor.matmul(pi[:, j, 0:CHW], lhsT=wliT,
                                         rhs=yrf[:, j, :], start=True, stop=False)
                        nc.tensor.matmul(pi[:, j, 0:CHW], lhsT=wlrT,
                                         rhs=yif[:, j, :], start=False, stop=True)
                    zr = zev.tile([128, NCHUNK, CHW], BF16, tag="zr", name=f"zr{f}")
                    zi = zev.tile([128, NCHUNK, CHW], BF16, tag="zi", name=f"zi{f}")
                    nc.scalar.activation(out=zr, in_=pr[:, :, 0:CHW], func=AF.Copy,
                                         accum_out=acc_s[:, f:f + 1])
                    nc.scalar.activation(out=zi, in_=pi[:, :, 0:CHW], func=AF.Copy,
                                         accum_out=acc_s[:, NF + f:NF + f + 1])
                    nc.scalar.activation(
                        out=sq_scr.rearrange("p (c x) -> p c x", c=NCHUNK),
                        in_=zr, func=AF.Square, accum_out=acc_q[:, f:f + 1])
                    nc.scalar.activation(
                        out=sq_scr.rearrange("p (c x) -> p c x", c=NCHUNK),
                        in_=zi, func=AF.Square,
                        accum_out=acc_q[:, NF + f:NF + f + 1])
                    nc.sync.dma_start(out=z_dram[:, f, :],
                                      in_=zr.rearrange("p c x -> p (c x)"))
                    nc.sync.dma_start(out=z_dram[:, NF + f, :],
                                      in_=zi.rearrange("p c x -> p (c x)"))
            nc.vector.tensor_reduce(out=stats[:, 0:1], in_=acc_s[:, 0:NF],
                                    axis=AX.X, op=AO.add)
            nc.vector.tensor_reduce(out=stats[:, 1:2], in_=acc_q[:, 0:NF],
                                    axis=AX.X, op=AO.add)
            nc.vector.tensor_reduce(out=stats[:, 2:3], in_=acc_s[:, NF:NQ],
                                    axis=AX.X, op=AO.add)
            nc.vector.tensor_reduce(out=stats[:, 3:4], in_=acc_q[:, NF:NQ],
                                    axis=AX.X, op=AO.add)

            # =========== Phase B4: AllReduce + BN affine terms =================
            cc_in = dram_p.tile([128, 4], F32)
            cc_out = dram_p.tile([128, 4], F32)
            nc.gpsimd.dma_start(out=cc_in[:], in_=stats)
            nc.gpsimd.collective_compute(
                "AllReduce", AO.add,
                replica_groups=[list(range(8))],
                ins=[cc_in.opt()], outs=[cc_out.opt()])
            nc.gpsimd.dma_start(out=rstats, in_=cc_out[:])
            inv_n = 1.0 / float(NBN)
            # kw: 0 mean_r, 1 mean_i, 2 var_r, 3 var_i, 4 sd_r, 5 sd_i,
            #     6 rs_r, 7 rs_i, 8 k1r, 9 k1i, 10 k2r, 11 k2i
            nc.vector.tensor_scalar_mul(kw[:, 0:1], rstats[:, 0:1], inv_n)
            nc.vector.tensor_scalar_mul(kw[:, 1:2], rstats[:, 2:3], inv_n)
            nc.vector.tensor_scalar_mul(kw[:, 2:3], rstats[:, 1:2], inv_n)
            nc.vector.tensor_scalar_mul(kw[:, 3:4], rstats[:, 3:4], inv_n)
            nc.vector.tensor_tensor(out=tmpk[:, 0:1], in0=kw[:, 0:1],
                                    in1=kw[:, 0:1], op=AO.mult)
            nc.vector.tensor_tensor(out=tmpk[:, 1:2], in0=kw[:, 1:2],
                                    in1=kw[:, 1:2], op=AO.mult)
            nc.vector.tensor_tensor(out=kw[:, 2:3], in0=kw[:, 2:3],
                                    in1=tmpk[:, 0:1], op=AO.subtract)
            nc.vector.tensor_tensor(out=kw[:, 3:4], in0=kw[:, 3:4],
                                    in1=tmpk[:, 1:2], op=AO.subtract)
            nc.scalar.activation(out=kw[:, 4:5], in_=kw[:, 2:3], func=AF.Sqrt,
                                 bias=epst)
            nc.scalar.activation(out=kw[:, 5:6], in_=kw[:, 3:4], func=AF.Sqrt,
                                 bias=epst)
            nc.vector.reciprocal(out=kw[:, 6:7], in_=kw[:, 4:5])
            nc.vector.reciprocal(out=kw[:, 7:8], in_=kw[:, 5:6])
            nc.vector.tensor_tensor(out=tmpk[:, 0:1], in0=kw[:, 6:7],
                                    in1=aff[:, 0:1], op=AO.mult)
            nc.vector.tensor_tensor(out=kw[:, 8:9], in0=tmpk[:, 0:1],
                                    in1=aff[:, 4:5], op=AO.mult)
            nc.vector.tensor_tensor(out=tmpk[:, 1:2], in0=kw[:, 7:8],
                                    in1=aff[:, 1:2], op=AO.mult)
            nc.vector.tensor_tensor(out=kw[:, 9:10], in0=tmpk[:, 1:2],
                                    in1=aff[:, 4:5], op=AO.mult)
            nc.vector.tensor_tensor(out=tmpk[:, 2:3], in0=kw[:, 0:1],
                                    in1=tmpk[:, 0:1], op=AO.mult)
            nc.vector.tensor_tensor(out=tmpk[:, 2:3], in0=aff[:, 2:3],
                                    in1=tmpk[:, 2:3], op=AO.subtract)
            nc.vector.tensor_tensor(out=kw[:, 10:11], in0=tmpk[:, 2:3],
                                    in1=aff[:, 4:5], op=AO.mult)
            nc.vector.tensor_tensor(out=tmpk[:, 3:4], in0=kw[:, 1:2],
                                    in1=tmpk[:, 1:2], op=AO.mult)
            nc.vector.tensor_tensor(out=tmpk[:, 3:4], in0=aff[:, 3:4],
                                    in1=tmpk[:, 3:4], op=AO.subtract)
            nc.vector.tensor_tensor(out=kw[:, 11:12], in0=tmpk[:, 3:4],
                                    in1=aff[:, 4:5], op=AO.mult)

            # =========== Phase B5: scale, irfft, final add =====================
            with tc.tile_pool(name="zld", bufs=2) as zld, \
                    tc.tile_pool(name="selp", bufs=2) as selp, \
                    tc.tile_pool(name="finp", bufs=3) as fin_p:
                for cc in range(NCC):
                    za = zld.tile([126, NCHUNK, CHW], BF16, tag="za", name=f"za{cc}")
                    nc.sync.dma_start(
                        out=za.rearrange("p c x -> p (c x)"),
                        in_=z_dram[cc * CL:cc * CL + 7].rearrange(
                            "o q x -> (o q) x"))
                    zb = zld.tile([18, NCHUNK, CHW], BF16, tag="zb", name=f"zb{cc}")
                    nc.sync.dma_start(
                        out=zb.rearrange("p c x -> p (c x)"),
                        in_=z_dram[cc * CL + 7])
                    k1s1 = selp.tile([128, 128], F32, tag="k1s1", name=f"k1s1_{cc}")
                    nc.sync.dma_start(out=k1s1, in_=k1r_d[cc])
                    k1s2 = selp.tile([128, 128], F32, tag="k1s2", name=f"k1s2_{cc}")
                    nc.sync.dma_start(out=k1s2, in_=k1i_d[cc])
                    k1s3 = selp.tile([128, 18], F32, tag="k1s3", name=f"k1s3_{cc}")
                    nc.sync.dma_start(out=k1s3, in_=k1br_d[cc])
                    k1s4 = selp.tile([128, 18], F32, tag="k1s4", name=f"k1s4_{cc}")
                    nc.sync.dma_start(out=k1s4, in_=k1bi_d[cc])
                    ds1 = selp.tile([128, 128], F32, tag="ds1", name=f"ds1_{cc}")
                    nc.sync.dma_start(out=ds1, in_=dsr_d[cc])
                    ds2 = selp.tile([128, 128], F32, tag="ds2", name=f"ds2_{cc}")
                    nc.sync.dma_start(out=ds2, in_=dsi_d[cc])
                    ds3 = selp.tile([128, 128], F32, tag="ds3", name=f"ds3_{cc}")
                    nc.sync.dma_start(out=ds3, in_=dsb_d[cc])
                    pk = ps.tile([128, NCHUNK, 512], F32, tag="ps", name=f"pk{cc}")
                    nc.tensor.matmul(pk[:, 0, 0:1], lhsT=k1s1, rhs=kw[:, 8:9],
                                     start=True, stop=False)
                    nc.tensor.matmul(pk[:, 0, 0:1], lhsT=k1s2, rhs=kw[:, 9:10],
                                     start=False, stop=True)
                    nc.tensor.matmul(pk[0:18, 1, 0:1], lhsT=k1s3, rhs=kw[:, 8:9],
                                     start=True, stop=False)
                    nc.tensor.matmul(pk[0:18, 1, 0:1], lhsT=k1s4, rhs=kw[:, 9:10],
                                     start=False, stop=True)
                    nc.tensor.matmul(pk[:, 2, 0:1], lhsT=ds1, rhs=kw[:, 10:11],
                                     start=True, stop=False)
                    nc.tensor.matmul(pk[:, 2, 0:1], lhsT=ds2, rhs=kw[:, 11:12],
                                     start=False, stop=False)
                    nc.tensor.matmul(pk[:, 2, 0:1], lhsT=ds3, rhs=aff[:, 5:6],
                                     start=False, stop=True)
                    zsa = zld.tile([126, NCHUNK, CHW], BF16, tag="zsa",
                                   name=f"zsa{cc}")
                    nc.vector.tensor_scalar_mul(zsa, za, pk[0:126, 0, 0:1])
                    zsb = zld.tile([18, NCHUNK, CHW], BF16, tag="zsb",
                                   name=f"zsb{cc}")
                    nc.vector.tensor_scalar_mul(zsb, zb, pk[0:18, 1, 0:1])
                    pf = ps.tile([128, NCHUNK, 512], F32, tag="ps", name=f"pfin{cc}")
                    for j in range(NCHUNK):
                        nc.tensor.matmul(pf[:, j, 0:CHW], lhsT=ira,
                                         rhs=zsa[:, j, :], start=True, stop=False)
                        nc.tensor.matmul(pf[:, j, 0:CHW], lhsT=irb,
                                         rhs=zsb[:, j, :], start=False, stop=True)
                    xid = fin_p.tile([128, NCHUNK, CHW], F32, tag="xid",
                                     name=f"xid{cc}")
                    nc.sync.dma_start(
                        out=xid.rearrange("p c x -> p (c x)"),
                        in_=x_id_d[cc].rearrange("p h w -> p (h w)"))
                    fin = fin_p.tile([128, NCHUNK, CHW], F32, tag="fin",
                                     name=f"fin{cc}")
                    nc.vector.scalar_tensor_tensor(
                        out=fin,
                        in0=pf[:, :, 0:CHW],
                        scalar=pk[:, 2, 0:1],
                        in1=xid,
                        op0=AO.add,
                        op1=AO.add)
                    nc.sync.dma_start(
                        out=out_d[cc].rearrange("p h w -> p (h w)"),
                        in_=fin.rearrange("p c x -> p (c x)"))
            ps_pool.__exit__(None, None, None)


    return nc


def _get_program():
    if "nc" not in _CACHE:
        _CACHE["nc"] = _build_program()
    return _CACHE["nc"]


# ------------------------------------------------------------------ host --
def _make_in_maps(inputs):
    x = np.asarray(inputs["x"], np.float32)
    consts = _build_consts(
        np.asarray(inputs["weights_cor"], np.float32),
        np.asarray(inputs["Wg"], np.float32),
        np.asarray(inputs["w1r"], np.float32),
        np.asarray(inputs["w1i"], np.float32),
        np.asarray(inputs["Wlr"], np.float32),
        np.asarray(inputs["Wli"], np.float32),
        np.asarray(inputs["gamma_r"], np.float32),
        np.asarray(inputs["beta_r"], np.float32),
        np.asarray(inputs["gamma_i"], np.float32),
        np.asarray(inputs["beta_i"], np.float32),
        np.asarray(inputs["alpha1"], np.float32),
        np.asarray(inputs["bias_p"], np.float32),
    )
    # padded full tensors (zero pad h by 1, w by 2 on each side)
    xp = np.pad(x, ((0, 0), (0, 0), (0, 0), (1, 1), (2, 2)))
    xn = np.concatenate([x[:, :, 1:], x[:, :, -1:]], axis=2)
    xnp = np.pad(xn, ((0, 0), (0, 0), (0, 0), (1, 1), (2, 2)))
    # w-shifted variants: reading var at w returns xn at w+dw
    xnp_m = np.zeros_like(xnp)
    xnp_m[..., 1:] = xnp[..., :-1]
    xnp_p = np.zeros_like(xnp)
    xnp_p[..., :-1] = xnp[..., 1:]

    def to_tc(a):
        # [C, T, h, w] -> [NCC, 128=(t*8+cl), h, w]
        hh, ww = a.shape[2], a.shape[3]
        return np.ascontiguousarray(
            a.reshape(NCC, CL, T, hh, ww).transpose(0, 2, 1, 3, 4)
            .reshape(NCC, 128, hh, ww))

    def to_ct(a):
        # [C, T, h, w] -> [NCC, 128=(cl*16+t), h, w]
        hh, ww = a.shape[2], a.shape[3]
        return np.ascontiguousarray(a.reshape(NCC, 128, hh, ww))

    in_maps = []
    for core in range(8):
        b, hf = core // 2, core % 2
        h0 = hf * Hh
        sl = np.s_[b, :, :, h0:h0 + HP, :]
        d = {
            "x_bf": to_tc(xp[sl]).astype(BF),
            "xn_c": to_tc(xnp[sl]).astype(BF),
            "xn_m": to_tc(xnp_m[sl]).astype(BF),
            "xn_p": to_tc(xnp_p[sl]).astype(BF),
            "x_id": to_ct(x[b, :, :, h0:h0 + Hh, :]).astype(np.float32),
        }
        d.update(consts)
        in_maps.append(d)
    return in_maps


def kernel(**inputs):
    in_maps = _make_in_maps(inputs)
    nc = _get_program()
    res = run_bass_kernel_spmd(nc, in_maps, core_ids=list(range(8)))
    out = np.zeros((B, C, T, H, W), np.float32)
    for core in range(8):
        b, hf = core // 2, core % 2
        # core result: [NCC, 128=(cl*16+t), Hh, W] -> [C, T, Hh, W]
        r = res.results[core]["out"].reshape(C, T, Hh, W)
        out[b, :, :, hf * Hh:(hf + 1) * Hh, :] = r
    return out



# revision 19
# speedup vs baseline: 1.2999x; 1.2999x over previous
"""DTFBlock Trainium2 kernel: 8-core SPMD (batch x H-half sharding).

Per-core layout: partition = (t*8 + c_local) over 8-channel chunks, free =
padded spatial frame [30, 60] (interior 28x56 at row 1 / col 2).  All
contractions run on PE via host-built block-diagonal / selection matrices;
3x3 shifts are free-axis AP offsets; the frame shift (t+1) is a partition
offset; BN statistics are AllReduced across the 8 cores.

v2: single fp16 upload of x (frame-shifted copy derived on device), fp16
everywhere, BN scales applied via tiny mask matmuls post-AllReduce, and the
f32 identity added on the host.
"""

import numpy as np

import bass_rust
import concourse.bass as bass
import concourse.tile as tile
from concourse import mybir
from concourse.bass_utils import run_bass_kernel_spmd

# --------------------------------------------------------------- patch ----
# This container's walrus rejects instructions carrying more than one sync
# wait: hoist extras into same-engine NOPs placed before the instruction.
_orig_sched = tile.TileContext.schedule_and_allocate


def _split_sync_waits(nc, max_waits=1):
    for f in nc.m.functions:
        for bb in f.blocks:
            il = list(bb.instructions)
            new = []
            changed = False
            for ins in il:
                si = ins.sync_info
                if si is not None and len(si.on_wait) > max_waits:
                    waits = list(si.on_wait)
                    for j, w in enumerate(waits[:-max_waits]):
                        nop = bass_rust.InstNoOp(
                            name=f"{ins.name}-ws{j}",
                            engine=ins.engine,
                            ins=[],
                            outs=[],
                            sync_info=bass_rust.SyncInfo(on_wait=[w], on_update=[]),
                        )
                        new.append(nop)
                        changed = True
                    ins.sync_info = bass_rust.SyncInfo(
                        on_wait=waits[-max_waits:], on_update=list(si.on_update)
                    )
                new.append(ins)
            if changed:
                bb.instructions = new


def _patched_sched(self, *a, **k):
    r = _orig_sched(self, *a, **k)
    _split_sync_waits(self.nc)
    return r


if tile.TileContext.schedule_and_allocate.__name__ != "_patched_sched":
    tile.TileContext.schedule_and_allocate = _patched_sched

# --------------------------------------------------------------- consts ---
B, C, T, H, W = 4, 128, 16, 56, 56
K = 3
NF = T // 2 + 1          # 9 rfft bins
KK = K * K
EPS = 1e-5
Hh = H // 2              # 28 rows per core
HP, WP = Hh + 2, W + 4   # padded frame 30 x 60
FR = HP * WP             # 1800
HWI = Hh * W             # 1568
CL = 8                   # channels per chunk
NCC = C // CL            # 16 chunks
NQ = 2 * NF              # 18 (ri, f); q = ri*9 + f
SOFF = 64                # s-frame margin
SFR = 2048               # s-frame row length
NBN = B * NF * H * W     # BN count per channel
F32 = mybir.dt.float32
F16 = mybir.dt.float16
H16 = np.float16
NCHUNK, CHH, CHW = 4, 7, 392  # hw chunks: 4 x (7 rows * 56)

DELTAS = [(ki - 1, kj - 1) for ki in range(K) for kj in range(K)]


def _dft_mats():
    Fm = np.fft.rfft(np.eye(T), axis=0, norm="ortho")  # [9, 16]
    Mr = np.zeros((T, NF))
    Mi = np.zeros((T, NF))
    for f in range(NF):
        e = np.zeros(NF, complex)
        e[f] = 1.0
        Mr[:, f] = np.fft.irfft(e, n=T, norm="ortho")
        Mi[:, f] = np.fft.irfft(1j * e, n=T, norm="ortho")
    return Fm.real.copy(), Fm.imag.copy(), Mr, Mi


def _build_consts(weights_cor, Wg, w1r, w1i, Wlr, Wli,
                  gamma_r, beta_r, gamma_i, beta_i, alpha1, bias_p):
    c = {}
    f4 = np.float32
    wc = np.zeros((128, KK, NCC, T), f4)
    for kk in range(KK):
        ki, kj = kk // K, kk % K
        for cc in range(NCC):
            for cl in range(CL):
                for t in range(T):
                    wc[t * CL + cl, kk, cc, t] = weights_cor[cc * CL + cl, t, ki, kj]
    c["wc"] = wc.astype(H16)
    # aggregation identity with the frame shift folded in:
    # out[m] = tau_sum[m + 8]  (t -> t+1 shift; rows 120..127 become 0)
    ish = np.zeros((128, 128), f4)
    for m in range(120):
        ish[m + 8, m] = 1.0
    c["ident_shift"] = ish.astype(H16)
    rep16 = np.zeros((T, 128), f4)
    for t in range(T):
        rep16[t, t * CL:(t + 1) * CL] = 1.0
    c["rep16"] = rep16.astype(H16)
    c["eye16"] = np.eye(T, dtype=f4).astype(H16)

    Fr, Fi, Mr, Mi = _dft_mats()
    rfr = np.zeros((128, CL * NF), f4)
    rfi = np.zeros((128, CL * NF), f4)
    for t in range(T):
        for cl in range(CL):
            for f in range(NF):
                rfr[t * CL + cl, cl * NF + f] = Fr[f, t]
                rfi[t * CL + cl, cl * NF + f] = Fi[f, t]
    c["rfftR"] = rfr.astype(H16)
    c["rfftI"] = rfi.astype(H16)

    wg_a = np.zeros((128, NCC, 128), f4)
    wg_b = np.zeros((128, NCC, 16), f4)
    for cc in range(NCC):
        for t in range(T):
            for cl in range(CL):
                col = (cc * CL + cl) * T + t
                wg_a[t * CL + cl, cc, :] = Wg[:128, col]
                wg_b[t * CL + cl, cc, :] = Wg[128:, col]
    c["wg_a"] = wg_a.astype(H16)
    c["wg_b"] = wg_b.astype(H16)
    wo_a = np.zeros((128, 128), f4)
    wo_b = np.zeros((128, 16), f4)
    wo2_a = np.zeros((16, 128), f4)
    wo2_b = np.zeros((16, 16), f4)
    for t in range(T):
        for kk in range(8):
            col = (C + kk) * T + t
            wo_a[t * 8 + kk, :] = Wg[:128, col]
            wo_b[t * 8 + kk, :] = Wg[128:, col]
        col = (C + 8) * T + t
        wo2_a[t, :] = Wg[:128, col]
        wo2_b[t, :] = Wg[128:, col]
    c["wg_wo_a"] = wo_a.astype(H16)
    c["wg_wo_b"] = wo_b.astype(H16)
    c["wg_wo2_a"] = wo2_a.astype(H16)
    c["wg_wo2_b"] = wo2_b.astype(H16)

    w1A = np.zeros((NCC, 144, CL * NF), f4)
    w1B = np.zeros((NCC, 144, CL * NF), f4)
    for cc in range(NCC):
        for cl in range(CL):
            ch = cc * CL + cl
            for f in range(NF):
                o_r = (18 * ch + 2 * f) // 16
                o_i = (18 * ch + 2 * f + 1) // 16
                m = cl * NF + f
                w1A[cc, o_r, m] += w1r[ch, f]
                w1A[cc, o_i, m] -= w1i[ch, f]
                w1B[cc, o_r, m] += w1i[ch, f]
                w1B[cc, o_i, m] += w1r[ch, f]
    # [144, NCC, 72] split at partition 128 for SBUF residency
    w1A = np.ascontiguousarray(w1A.transpose(1, 0, 2))
    w1B = np.ascontiguousarray(w1B.transpose(1, 0, 2))
    c["w1A_p1"] = w1A[:128].astype(H16)
    c["w1A_p2"] = w1A[128:].astype(H16)
    c["w1B_p1"] = w1B[:128].astype(H16)
    c["w1B_p2"] = w1B[128:].astype(H16)
    c["wlrT"] = np.ascontiguousarray(Wlr.T).astype(H16)
    c["wliT"] = np.ascontiguousarray(Wli.T).astype(H16)
    c["wliTn"] = np.ascontiguousarray(-Wli.T).astype(H16)

    # irfft block-diag over z-chunk rows k = cl*18 + q; K split 126 + 18
    Mhat = np.concatenate([Mr, Mi], axis=1)  # [16, 18]
    ir_a = np.zeros((126, 128), f4)
    ir_b = np.zeros((18, 128), f4)
    for cl in range(7):
        for q in range(NQ):
            ir_a[cl * NQ + q, cl * T:(cl + 1) * T] = Mhat[:, q]
    for q in range(NQ):
        ir_b[q, 7 * T:8 * T] = Mhat[:, q]
    c["irfft_a"] = ir_a.astype(H16)
    c["irfft_b"] = ir_b.astype(H16)

    # BN-scale routing consts (replace the fat f32 selection matmuls):
    # mask16[o, cc] = 1 if o // 8 == cc  (chunk membership of channel o)
    # SELP[o, m=(cl,t)] = 1 if o % 8 == cl        -> per-(cl,t) gather
    # D_A / D_B add the irfft-of-constant factors SMr/SMi per t
    # SELQ_A/B[o, m=(cl,q)|(126+q)] route k1r (q<9) / k1i (q>=9) to z rows
    mask16 = np.zeros((128, NCC), f4)
    for o in range(128):
        mask16[o, o // 8] = 1.0
    c["mask16"] = mask16.astype(H16)
    SMr = Mr.sum(axis=1)
    SMi = Mi.sum(axis=1)
    selp = np.zeros((128, 128), f4)
    d_a = np.zeros((128, 128), f4)
    d_b = np.zeros((128, 128), f4)
    for o in range(128):
        cl = o % 8
        for t in range(T):
            m = cl * T + t
            selp[o, m] = 1.0
            d_a[o, m] = SMr[t]
            d_b[o, m] = SMi[t]
    c["selp"] = selp.astype(H16)
    c["d_a"] = d_a.astype(H16)
    c["d_b"] = d_b.astype(H16)
    selq_a = np.zeros((128, 144), f4)
    selq_b = np.zeros((128, 144), f4)
    for o in range(128):
        cl = o % 8
        for q in range(NQ):
            m = cl * NQ + q if cl < 7 else 126 + q
            (selq_a if q < NF else selq_b)[o, m] = 1.0
    c["selq_a"] = selq_a.astype(H16)
    c["selq_b"] = selq_b.astype(H16)

    aff = np.zeros((128, 8), f4)
    aff[:, 0] = gamma_r
    aff[:, 1] = gamma_i
    aff[:, 2] = beta_r
    aff[:, 3] = beta_i
    aff[:, 4] = alpha1.reshape(-1)
    aff[:, 5] = bias_p.reshape(-1)
    c["aff"] = aff
    return c


# ---------------------------------------------------------- bass program --
_CACHE = {}

AO = mybir.AluOpType
AX = mybir.AxisListType
AF = mybir.ActivationFunctionType


def _build_program():
    nc = bass.Bass("TRN2", target_bir_lowering=False, debug=False, num_devices=8)

    def din(name, shape, dtype=F16):
        return nc.dram_tensor(name, list(shape), dtype, kind="ExternalInput").ap()

    x_d = din("x_p", [128, NCC, HP, WP])
    wc_d = din("wc", [128, KK, NCC, T])
    ish_d = din("ident_shift", [128, 128])
    rep16_d = din("rep16", [T, 128])
    eye16_d = din("eye16", [T, T])
    rfr_d = din("rfftR", [128, CL * NF])
    rfi_d = din("rfftI", [128, CL * NF])
    wga_d = din("wg_a", [128, NCC, 128])
    wgb_d = din("wg_b", [128, NCC, 16])
    wgwoa_d = din("wg_wo_a", [128, 128])
    wgwob_d = din("wg_wo_b", [128, 16])
    wgwo2a_d = din("wg_wo2_a", [16, 128])
    wgwo2b_d = din("wg_wo2_b", [16, 16])
    w1a1_d = din("w1A_p1", [128, NCC, CL * NF])
    w1a2_d = din("w1A_p2", [16, NCC, CL * NF])
    w1b1_d = din("w1B_p1", [128, NCC, CL * NF])
    w1b2_d = din("w1B_p2", [16, NCC, CL * NF])
    wlrT_d = din("wlrT", [128, 128])
    wliT_d = din("wliT", [128, 128])
    wliTn_d = din("wliTn", [128, 128])
    ira_d = din("irfft_a", [126, 128])
    irb_d = din("irfft_b", [18, 128])
    mask16_d = din("mask16", [128, NCC])
    selp_d = din("selp", [128, 128])
    da_d = din("d_a", [128, 128])
    db_d = din("d_b", [128, 128])
    selqa_d = din("selq_a", [128, 144])
    selqb_d = din("selq_b", [128, 144])
    aff_d = din("aff", [128, 8], F32)

    out_d = nc.dram_tensor("out", [NCC, 128, Hh, W], F16,
                           kind="ExternalOutput").ap()
    y_dram = nc.dram_tensor("y_stage", [2, NCC, CL, NF, HWI], F16,
                            kind="Internal").ap()
    z_dram = nc.dram_tensor("z_stage", [C, NQ, HWI], F16, kind="Internal").ap()
    wo_dram = nc.dram_tensor("wo_stage", [16, 8, HWI], F16, kind="Internal").ap()
    wo2_dram = nc.dram_tensor("wo2_stage", [16, HWI], F16, kind="Internal").ap()

    from contextlib import ExitStack

    with tile.TileContext(nc) as tc, ExitStack() as ctx:
        cst = ctx.enter_context(tc.tile_pool(name="cst", bufs=1))
        sfp = ctx.enter_context(tc.tile_pool(name="sfp", bufs=1))
        dram_p = ctx.enter_context(tc.tile_pool(name="dramp", bufs=2, space="DRAM"))

        # ---- consts to SBUF
        def cload(tag, d, shape, dtype=F16):
            t = cst.tile(list(shape), dtype, tag=tag)
            nc.sync.dma_start(out=t, in_=d)
            return t

        wc_sb = cload("wc", wc_d, [128, KK, NCC, T])
        ish = cload("ish", ish_d, [128, 128])
        rep16 = cload("rep16", rep16_d, [T, 128])
        eye16 = cload("eye16", eye16_d, [T, T])
        rfr = cload("rfr", rfr_d, [128, CL * NF])
        rfi = cload("rfi", rfi_d, [128, CL * NF])
        wga = cload("wga", wga_d, [128, NCC, 128])
        wgb = cload("wgb", wgb_d, [128, NCC, 16])
        wgwoa = cload("wgwoa", wgwoa_d, [128, 128])
        wgwob = cload("wgwob", wgwob_d, [128, 16])
        wgwo2a = cload("wgwo2a", wgwo2a_d, [16, 128])
        wgwo2b = cload("wgwo2b", wgwo2b_d, [16, 16])
        w1a1 = cload("w1a1", w1a1_d, [128, NCC, CL * NF])
        w1a2 = cload("w1a2", w1a2_d, [16, NCC, CL * NF])
        w1b1 = cload("w1b1", w1b1_d, [128, NCC, CL * NF])
        w1b2 = cload("w1b2", w1b2_d, [16, NCC, CL * NF])
        wlrT = cload("wlrT", wlrT_d, [128, 128])
        wliT = cload("wliT", wliT_d, [128, 128])
        wliTn = cload("wliTn", wliTn_d, [128, 128])
        ira = cload("ira", ira_d, [126, 128])
        irb = cload("irb", irb_d, [18, 128])
        mask16 = cload("mask16", mask16_d, [128, NCC])
        selp = cload("selp", selp_d, [128, 128])
        d_a = cload("d_a", da_d, [128, 128])
        d_b = cload("d_b", db_d, [128, 128])
        selq_a = cload("selq_a", selqa_d, [128, 144])
        selq_b = cload("selq_b", selqb_d, [128, 144])
        aff = cload("aff", aff_d, [128, 8], F32)

        # ---- persistent small working tensors
        acc_s = sfp.tile([128, 2 * NF * NCHUNK], F32, tag="acc_s")
        acc_q = sfp.tile([128, 2 * NF * NCHUNK], F32, tag="acc_q")
        stats = sfp.tile([128, 4], F32, tag="stats")
        rstats = sfp.tile([128, 4], F32, tag="rstats")
        kw = sfp.tile([128, 12], F32, tag="kw")
        tmpk = sfp.tile([128, 4], F32, tag="tmpk")
        epst = sfp.tile([128, 1], F32, tag="epst")
        nc.vector.memset(epst, EPS)
        # BN-scale vectors (built post-AllReduce)
        kmask = sfp.tile([128, 4, NCC], F16, tag="kmask")
        zvec_a = sfp.tile([126, NCC], F32, tag="zvec_a")
        zvec_b = sfp.tile([18, NCC], F32, tag="zvec_b")
        dterm = sfp.tile([128, NCC], F32, tag="dterm")

        # =========== Phase 1: correlation ==================================
        x2_pool = tc.tile_pool(name="x2p", bufs=1)
        x2p = x2_pool.__enter__()
        srep_pool = tc.tile_pool(name="srep", bufs=1)
        srp = srep_pool.__enter__()
        sf_pool = tc.tile_pool(name="sfpool", bufs=1)
        sfE = sf_pool.__enter__()
        sframes = []
        for kk in range(KK):
            sf = sfE.tile([16, SFR], F16, tag=f"sf{kk}", name=f"sf{kk}")
            nc.gpsimd.memset(sf, 0.0)
            sframes.append(sf)
        rD16 = sfE.tile([16, NCHUNK, CHW], F16, tag="rD16")

        def sf_int(kk, j=None):
            v = sframes[kk][:, SOFF:SOFF + FR].rearrange(
                "p (h w) -> p h w", h=HP)
            if j is None:
                return v[:, 1:1 + Hh, 2:2 + W]
            return v[:, 1 + CHH * j:1 + CHH * (j + 1), 2:2 + W]

        with tc.tile_pool(name="wops", bufs=1, space="PSUM") as wop, \
                tc.tile_pool(name="xst", bufs=3) as xst, \
                tc.tile_pool(name="xnt", bufs=3) as xn_p, \
                tc.tile_pool(name="wtp", bufs=2) as wt_p, \
                tc.tile_pool(name="vp", bufs=3) as v_p:
            wo_ps = [wop.tile([128, 512], F32, tag=f"wops{kk}", name=f"wops{kk}")
                     for kk in range(8)]

            def corr_mm(pst, kk, cc, v, first, last):
                for j in range(NCHUNK):
                    nc.tensor.matmul(
                        pst[32 * j:32 * j + 16, 0:CHW],
                        lhsT=wc_sb[:, kk, cc, :],
                        rhs=v[:, CHH * j:CHH * (j + 1), :],
                        start=first,
                        stop=last,
                        tile_position=(0, 32 * j),
                    )

            def evict_wo(pst, kk):
                wt = wt_p.tile([16, Hh, W], F16, tag="wt", name=f"wt{kk}")
                for j in range(NCHUNK):
                    pin = pst[32 * j:32 * j + 16, 0:CHW].rearrange(
                        "p (hh w) -> p hh w", hh=CHH)
                    nc.scalar.activation(
                        out=wt[:, CHH * j:CHH * (j + 1), :],
                        in_=pin, func=AF.Copy)
                nc.scalar.activation(out=sf_int(kk), in_=wt, func=AF.Exp)
                if kk < 8:
                    nc.sync.dma_start(out=wo_dram[:, kk, :],
                                      in_=wt.rearrange("p h w -> p (h w)"))
                else:
                    nc.sync.dma_start(out=wo2_dram,
                                      in_=wt.rearrange("p h w -> p (h w)"))

            def load_x(cc, tag):
                xt = xst.tile([128, HP, WP], F16, tag="xs", name=f"xs{tag}_{cc}")
                nc.sync.dma_start(out=xt, in_=x_d[:, cc])
                return xt

            def load_xn(cc, tag):
                xn = xn_p.tile([128, HP, WP], F16, tag="xn",
                               name=f"xn{tag}_{cc}")
                nc.sync.dma_start(out=xn[0:120], in_=x_d[8:128, cc])
                nc.sync.dma_start(out=xn[120:128], in_=x_d[120:128, cc])
                return xn

            for cc in range(NCC):
                xt = load_x(cc, "a")
                xn = load_xn(cc, "a")
                for kk in range(8):
                    dh, dw = DELTAS[kk]
                    v = v_p.tile([128, Hh, W], F16, tag="v", name=f"v{cc}_{kk}")
                    nc.vector.tensor_tensor(
                        out=v,
                        in0=xt[:, 1:1 + Hh, 2:2 + W],
                        in1=xn[:, 1 + dh:1 + dh + Hh, 2 + dw:2 + dw + W],
                        op=AO.mult,
                    )
                    corr_mm(wo_ps[kk], kk, cc, v, cc == 0, cc == NCC - 1)
            for kk in range(8):
                evict_wo(wo_ps[kk], kk)
            # kk8 = (1, 1): reuse the wops0 bank
            p8 = wop.tile([128, 512], F32, tag="wops0", name="wops8")
            for cc in range(NCC):
                xt = load_x(cc, "b")
                xn = load_xn(cc, "b")
                v = v_p.tile([128, Hh, W], F16, tag="v", name=f"v8_{cc}")
                nc.vector.tensor_tensor(
                    out=v,
                    in0=xt[:, 1:1 + Hh, 2:2 + W],
                    in1=xn[:, 2:2 + Hh, 3:3 + W],
                    op=AO.mult,
                )
                corr_mm(p8, 8, cc, v, cc == 0, cc == NCC - 1)
            evict_wo(p8, 8)
        # ---- softmax: denominators + normalize the frames in place
        ps_pool = tc.tile_pool(name="ps", bufs=2, space="PSUM")
        ps = ps_pool.__enter__()
        pd = ps.tile([128, NCHUNK, 512], F32, tag="ps", name="pdsum")
        for j in range(NCHUNK):
            for kk in range(KK):
                nc.tensor.matmul(
                    pd[0:16, j, 0:CHW], lhsT=eye16,
                    rhs=sf_int(kk, j), start=(kk == 0), stop=(kk == KK - 1))
        with nc.allow_low_precision(reason="softmax denominators ~9, fp16 ok"):
            nc.vector.reciprocal(out=rD16, in_=pd[0:16, :, 0:CHW])
        for kk in range(KK):
            nc.vector.tensor_tensor(
                out=sf_int(kk),
                in0=sf_int(kk),
                in1=rD16.rearrange("p c (hh w) -> p (c hh) w", hh=CHH),
                op=AO.mult)

        # =========== Phase 1c: s' replication ==============================
        NS = 450
        srep = []
        for kk in range(KK):
            dh, dw = DELTAS[kk]
            delta = dh * WP + dw
            p = ps.tile([128, NCHUNK, 512], F32, tag="ps", name=f"psrep{kk}")
            for m in range(NCHUNK):
                nc.tensor.matmul(
                    p[:, m, 0:NS], lhsT=rep16,
                    rhs=sframes[kk][:, SOFF - delta + NS * m:
                                    SOFF - delta + NS * (m + 1)],
                    start=True, stop=True)
            st = srp.tile([128, FR], F16, tag=f"srep{kk}", name=f"srep{kk}")
            nc.scalar.activation(
                out=st.rearrange("p (c x) -> p c x", c=NCHUNK),
                in_=p[:, :, 0:NS], func=AF.Copy)
            srep.append(st)
        sf_pool.__exit__(None, None, None)

        # =========== Phase 2: aggregation + x2 =============================
        x2 = []
        with tc.tile_pool(name="xst2", bufs=3) as xst2, \
                tc.tile_pool(name="taup", bufs=3) as tau_p:
            for cc in range(NCC):
                xt = xst2.tile([128, HP, WP], F16, tag="xs2", name=f"xs2_{cc}")
                nc.sync.dma_start(out=xt, in_=x_d[:, cc])
                agg = ps.tile([128, NCHUNK, 512], F32, tag="ps", name=f"agg{cc}")
                for kk in range(KK):
                    dh, dw = DELTAS[kk]
                    tau = tau_p.tile([128, HP, WP], F16, tag="tau",
                                     name=f"tau{cc}_{kk}")
                    nc.vector.tensor_tensor(
                        out=tau.rearrange("p h w -> p (h w)"),
                        in0=srep[kk],
                        in1=xt.rearrange("p h w -> p (h w)"),
                        op=AO.mult)
                    for j in range(NCHUNK):
                        nc.tensor.matmul(
                            agg[:, j, 0:CHW],
                            lhsT=ish,
                            rhs=tau[:, 1 + CHH * j + dh:1 + CHH * (j + 1) + dh,
                                    2 + dw:2 + dw + W],
                            start=(kk == 0),
                            stop=(kk == KK - 1))
                xv = x2p.tile([128, NCHUNK, CHW], F16, tag=f"x2{cc}",
                              name=f"x2{cc}")
                # x2 = a_shift + x  (shift already folded into ish)
                nc.vector.tensor_tensor(
                    out=xv.rearrange("p c (hh w) -> p c hh w", hh=CHH),
                    in0=agg[:, :, 0:CHW].rearrange(
                        "p c (hh w) -> p c hh w", hh=CHH),
                    in1=xt[:, 1:1 + Hh, 2:2 + W].rearrange(
                        "p (c hh) w -> p c hh w", hh=CHH),
                    op=AO.add)
                x2.append(xv)
        srep_pool.__exit__(None, None, None)

        # w_o tiles for the Wg input (loaded from the DRAM staging copy)
        itp_pool = tc.tile_pool(name="itp", bufs=1)
        itp = itp_pool.__enter__()
        wo_t1 = itp.tile([128, NCHUNK, CHW], F16, tag="wo_t1")
        wo_t2 = itp.tile([16, NCHUNK, CHW], F16, tag="wo_t2")
        it_a = itp.tile([128, NCHUNK, CHW], F16, tag="it_a")
        it_b = itp.tile([16, NCHUNK, CHW], F16, tag="it_b")
        nc.sync.dma_start(
            out=wo_t1.rearrange("p c w -> p (c w)"),
            in_=wo_dram.rearrange("t k x -> (t k) x"))
        nc.sync.dma_start(
            out=wo_t2.rearrange("p c w -> p (c w)"),
            in_=wo2_dram)

        # =========== Phase B1: Wg 1x1 conv -> iterm ========================
        for j in range(NCHUNK):
            pg1 = ps.tile([128, NCHUNK, 512], F32, tag="ps", name=f"pg1_{j}")
            for cc in range(NCC):
                nc.tensor.matmul(pg1[:, 0, 0:CHW], lhsT=wga[:, cc, :],
                                 rhs=x2[cc][:, j, :], start=(cc == 0), stop=False)
            nc.tensor.matmul(pg1[:, 0, 0:CHW], lhsT=wgwoa,
                             rhs=wo_t1[:, j, :], start=False, stop=False)
            nc.tensor.matmul(pg1[:, 0, 0:CHW], lhsT=wgwo2a,
                             rhs=wo_t2[:, j, :], start=False, stop=True)
            for cc in range(NCC):
                nc.tensor.matmul(pg1[0:16, 1, 0:CHW], lhsT=wgb[:, cc, :],
                                 rhs=x2[cc][:, j, :], start=(cc == 0), stop=False)
            nc.tensor.matmul(pg1[0:16, 1, 0:CHW], lhsT=wgwob,
                             rhs=wo_t1[:, j, :], start=False, stop=False)
            nc.tensor.matmul(pg1[0:16, 1, 0:CHW], lhsT=wgwo2b,
                             rhs=wo_t2[:, j, :], start=False, stop=True)
            nc.scalar.activation(out=it_a[:, j, :], in_=pg1[:, 0, 0:CHW],
                                 func=AF.Copy)
            nc.scalar.activation(out=it_b[:, j, :], in_=pg1[0:16, 1, 0:CHW],
                                 func=AF.Copy)

        # =========== Phase B2: rfft + dynamic filter + pointwise ==========
        with tc.tile_pool(name="pwp", bufs=2) as pw_p:
            for cc in range(NCC):
                pr = ps.tile([128, NCHUNK, 512], F32, tag="ps", name=f"prf{cc}")
                pi = ps.tile([128, NCHUNK, 512], F32, tag="ps", name=f"pif{cc}")
                for j in range(NCHUNK):
                    nc.tensor.matmul(pr[0:CL * NF, j, 0:CHW], lhsT=rfr,
                                     rhs=x2[cc][:, j, :], start=True, stop=True)
                    nc.tensor.matmul(pi[0:CL * NF, j, 0:CHW], lhsT=rfi,
                                     rhs=x2[cc][:, j, :], start=True, stop=True)
                xtr = pw_p.tile([CL * NF, NCHUNK, CHW], F16, tag="xtr",
                                name=f"xtr{cc}")
                xti = pw_p.tile([CL * NF, NCHUNK, CHW], F16, tag="xti",
                                name=f"xti{cc}")
                nc.scalar.activation(out=xtr, in_=pr[0:CL * NF, :, 0:CHW],
                                     func=AF.Copy)
                nc.scalar.activation(out=xti, in_=pi[0:CL * NF, :, 0:CHW],
                                     func=AF.Copy)
                pa = ps.tile([128, NCHUNK, 512], F32, tag="ps", name=f"pa{cc}")
                pb = ps.tile([128, NCHUNK, 512], F32, tag="ps", name=f"pb{cc}")
                for j in range(NCHUNK):
                    nc.tensor.matmul(pa[0:CL * NF, j, 0:CHW],
                                     lhsT=w1a1[:, cc, :],
                                     rhs=it_a[:, j, :], start=True, stop=False)
                    nc.tensor.matmul(pa[0:CL * NF, j, 0:CHW],
                                     lhsT=w1a2[:, cc, :],
                                     rhs=it_b[:, j, :], start=False, stop=True)
                    nc.tensor.matmul(pb[0:CL * NF, j, 0:CHW],
                                     lhsT=w1b1[:, cc, :],
                                     rhs=it_a[:, j, :], start=True, stop=False)
                    nc.tensor.matmul(pb[0:CL * NF, j, 0:CHW],
                                     lhsT=w1b2[:, cc, :],
                                     rhs=it_b[:, j, :], start=False, stop=True)
                Am = pw_p.tile([CL * NF, NCHUNK, CHW], F16, tag="Am",
                               name=f"Am{cc}")
                Bm = pw_p.tile([CL * NF, NCHUNK, CHW], F16, tag="Bm",
                               name=f"Bm{cc}")
                nc.scalar.activation(out=Am, in_=pa[0:CL * NF, :, 0:CHW],
                                     func=AF.Copy)
                nc.scalar.activation(out=Bm, in_=pb[0:CL * NF, :, 0:CHW],
                                     func=AF.Copy)
                t1 = pw_p.tile([CL * NF, NCHUNK, CHW], F16, tag="t1",
                               name=f"t1{cc}")
                t2 = pw_p.tile([CL * NF, NCHUNK, CHW], F16, tag="t2",
                               name=f"t2{cc}")
                yr = pw_p.tile([CL * NF, NCHUNK, CHW], F16, tag="yr",
                               name=f"yrt{cc}")
                yi = pw_p.tile([CL * NF, NCHUNK, CHW], F16, tag="yi",
                               name=f"yit{cc}")
                nc.vector.tensor_tensor(out=t1, in0=xtr, in1=Am, op=AO.mult)
                nc.vector.tensor_tensor(out=t2, in0=xti, in1=Bm, op=AO.mult)
                nc.vector.tensor_tensor(out=yr, in0=t1, in1=t2, op=AO.subtract)
                nc.vector.tensor_tensor(out=t1, in0=xtr, in1=Bm, op=AO.mult)
                nc.vector.tensor_tensor(out=t2, in0=xti, in1=Am, op=AO.mult)
                nc.vector.tensor_tensor(out=yi, in0=t1, in1=t2, op=AO.add)
                nc.sync.dma_start(
                    out=y_dram[0, cc].rearrange("c f x -> (c f) x"),
                    in_=yr.rearrange("p c x -> p (c x)"))
                nc.sync.dma_start(
                    out=y_dram[1, cc].rearrange("c f x -> (c f) x"),
                    in_=yi.rearrange("p c x -> p (c x)"))
        itp_pool.__exit__(None, None, None)
        x2_pool.__exit__(None, None, None)

        # =========== Phase B3: FFTLinear + BN stats ========================
        with tc.tile_pool(name="yld", bufs=3) as yld, \
                tc.tile_pool(name="zev", bufs=3) as zev:
            sq_scr = zev.tile([128, CHW], F16, tag="sq_scr", bufs=1)
            for f in range(NF):
                yrf = yld.tile([128, NCHUNK, CHW], F16, tag="yrf",
                               name=f"yrf{f}")
                nc.sync.dma_start(
                    out=yrf.rearrange("p c x -> p (c x)"),
                    in_=y_dram[0, :, :, f, :].rearrange("n c x -> (n c) x"))
                yif = yld.tile([128, NCHUNK, CHW], F16, tag="yif",
                               name=f"yif{f}")
                nc.sync.dma_start(
                    out=yif.rearrange("p c x -> p (c x)"),
                    in_=y_dram[1, :, :, f, :].rearrange("n c x -> (n c) x"))
                pr = ps.tile([128, NCHUNK, 512], F32, tag="ps", name=f"pfr{f}")
                pi = ps.tile([128, NCHUNK, 512], F32, tag="ps", name=f"pfi{f}")
                for j in range(NCHUNK):
                    nc.tensor.matmul(pr[:, j, 0:CHW], lhsT=wlrT,
                                     rhs=yrf[:, j, :], start=True, stop=False)
                    nc.tensor.matmul(pr[:, j, 0:CHW], lhsT=wliTn,
                                     rhs=yif[:, j, :], start=False, stop=True)
                    nc.tensor.matmul(pi[:, j, 0:CHW], lhsT=wliT,
                                     rhs=yrf[:, j, :], start=True, stop=False)
                    nc.tensor.matmul(pi[:, j, 0:CHW], lhsT=wlrT,
                                     rhs=yif[:, j, :], start=False, stop=True)
                zr = zev.tile([128, NCHUNK, CHW], F16, tag="zr", name=f"zr{f}")
                zi = zev.tile([128, NCHUNK, CHW], F16, tag="zi", name=f"zi{f}")
                for j in range(NCHUNK):
                    ir_ = f * NCHUNK + j
                    ii_ = (NF + f) * NCHUNK + j
                    nc.scalar.activation(out=zr[:, j, :], in_=pr[:, j, 0:CHW],
                                         func=AF.Copy,
                                         accum_out=acc_s[:, ir_:ir_ + 1])
                    nc.scalar.activation(out=zi[:, j, :], in_=pi[:, j, 0:CHW],
                                         func=AF.Copy,
                                         accum_out=acc_s[:, ii_:ii_ + 1])
                    nc.scalar.activation(
                        out=sq_scr, in_=zr[:, j, :], func=AF.Square,
                        accum_out=acc_q[:, ir_:ir_ + 1])
                    nc.scalar.activation(
                        out=sq_scr, in_=zi[:, j, :], func=AF.Square,
                        accum_out=acc_q[:, ii_:ii_ + 1])
                nc.sync.dma_start(out=z_dram[:, f, :],
                                  in_=zr.rearrange("p c x -> p (c x)"))
                nc.sync.dma_start(out=z_dram[:, NF + f, :],
                                  in_=zi.rearrange("p c x -> p (c x)"))
        NH = NF * NCHUNK
        nc.vector.tensor_reduce(out=stats[:, 0:1], in_=acc_s[:, 0:NH],
                                axis=AX.X, op=AO.add)
        nc.vector.tensor_reduce(out=stats[:, 1:2], in_=acc_q[:, 0:NH],
                                axis=AX.X, op=AO.add)
        nc.vector.tensor_reduce(out=stats[:, 2:3], in_=acc_s[:, NH:2 * NH],
                                axis=AX.X, op=AO.add)
        nc.vector.tensor_reduce(out=stats[:, 3:4], in_=acc_q[:, NH:2 * NH],
                                axis=AX.X, op=AO.add)

        # =========== Phase B4: AllReduce + BN affine terms =================
        cc_in = dram_p.tile([128, 4], F32)
        cc_out = dram_p.tile([128, 4], F32)
        nc.gpsimd.dma_start(out=cc_in[:], in_=stats)
        nc.gpsimd.collective_compute(
            "AllReduce", AO.add,
            replica_groups=[list(range(8))],
            ins=[cc_in.opt()], outs=[cc_out.opt()])
        nc.gpsimd.dma_start(out=rstats, in_=cc_out[:])
        inv_n = 1.0 / float(NBN)
        # kw: 0 mean_r, 1 mean_i, 2 var_r, 3 var_i, 4 sd_r, 5 sd_i,
        #     6 rs_r, 7 rs_i, 8 k1r, 9 k1i, 10 k2r, 11 k2i
        nc.vector.tensor_scalar_mul(kw[:, 0:1], rstats[:, 0:1], inv_n)
        nc.vector.tensor_scalar_mul(kw[:, 1:2], rstats[:, 2:3], inv_n)
        nc.vector.tensor_scalar_mul(kw[:, 2:3], rstats[:, 1:2], inv_n)
        nc.vector.tensor_scalar_mul(kw[:, 3:4], rstats[:, 3:4], inv_n)
        nc.vector.tensor_tensor(out=tmpk[:, 0:1], in0=kw[:, 0:1],
                                in1=kw[:, 0:1], op=AO.mult)
        nc.vector.tensor_tensor(out=tmpk[:, 1:2], in0=kw[:, 1:2],
                                in1=kw[:, 1:2], op=AO.mult)
        nc.vector.tensor_tensor(out=kw[:, 2:3], in0=kw[:, 2:3],
                                in1=tmpk[:, 0:1], op=AO.subtract)
        nc.vector.tensor_tensor(out=kw[:, 3:4], in0=kw[:, 3:4],
                                in1=tmpk[:, 1:2], op=AO.subtract)
        nc.scalar.activation(out=kw[:, 4:5], in_=kw[:, 2:3], func=AF.Sqrt,
                             bias=epst)
        nc.scalar.activation(out=kw[:, 5:6], in_=kw[:, 3:4], func=AF.Sqrt,
                             bias=epst)
        nc.vector.reciprocal(out=kw[:, 6:7], in_=kw[:, 4:5])
        nc.vector.reciprocal(out=kw[:, 7:8], in_=kw[:, 5:6])
        nc.vector.tensor_tensor(out=tmpk[:, 0:1], in0=kw[:, 6:7],
                                in1=aff[:, 0:1], op=AO.mult)
        nc.vector.tensor_tensor(out=kw[:, 8:9], in0=tmpk[:, 0:1],
                                in1=aff[:, 4:5], op=AO.mult)
        nc.vector.tensor_tensor(out=tmpk[:, 1:2], in0=kw[:, 7:8],
                                in1=aff[:, 1:2], op=AO.mult)
        nc.vector.tensor_tensor(out=kw[:, 9:10], in0=tmpk[:, 1:2],
                                in1=aff[:, 4:5], op=AO.mult)
        nc.vector.tensor_tensor(out=tmpk[:, 2:3], in0=kw[:, 0:1],
                                in1=tmpk[:, 0:1], op=AO.mult)
        nc.vector.tensor_tensor(out=tmpk[:, 2:3], in0=aff[:, 2:3],
                                in1=tmpk[:, 2:3], op=AO.subtract)
        nc.vector.tensor_tensor(out=kw[:, 10:11], in0=tmpk[:, 2:3],
                                in1=aff[:, 4:5], op=AO.mult)
        nc.vector.tensor_tensor(out=tmpk[:, 3:4], in0=kw[:, 1:2],
                                in1=tmpk[:, 1:2], op=AO.mult)
        nc.vector.tensor_tensor(out=tmpk[:, 3:4], in0=aff[:, 3:4],
                                in1=tmpk[:, 3:4], op=AO.subtract)
        nc.vector.tensor_tensor(out=kw[:, 11:12], in0=tmpk[:, 3:4],
                                in1=aff[:, 4:5], op=AO.mult)

        # ---- route k1r/k1i/k2r/k2i/bias_p through tiny mask matmuls
        # kmask[:, 0] = k1r masked per chunk, 1 = k1i, 2 = k2r, 3 = k2i
        nc.vector.tensor_scalar_mul(kmask[:, 0, :], mask16, kw[:, 8:9])
        nc.vector.tensor_scalar_mul(kmask[:, 1, :], mask16, kw[:, 9:10])
        nc.vector.tensor_scalar_mul(kmask[:, 2, :], mask16, kw[:, 10:11])
        nc.vector.tensor_scalar_mul(kmask[:, 3, :], mask16, kw[:, 11:12])
        bpm = sfp.tile([128, NCC], F16, tag="bpm")
        nc.vector.tensor_scalar_mul(bpm, mask16, aff[:, 5:6])
        pk = ps.tile([128, NCHUNK, 512], F32, tag="ps", name="pkvec")
        # zvec_a rows (cl*18+q), cl<7; zvec_b rows q (cl=7 block)
        nc.tensor.matmul(pk[0:126, 0, 0:NCC], lhsT=selq_a[:, 0:126],
                         rhs=kmask[:, 0, :], start=True, stop=False)
        nc.tensor.matmul(pk[0:126, 0, 0:NCC], lhsT=selq_b[:, 0:126],
                         rhs=kmask[:, 1, :], start=False, stop=True)
        nc.tensor.matmul(pk[0:18, 1, 0:NCC], lhsT=selq_a[:, 126:144],
                         rhs=kmask[:, 0, :], start=True, stop=False)
        nc.tensor.matmul(pk[0:18, 1, 0:NCC], lhsT=selq_b[:, 126:144],
                         rhs=kmask[:, 1, :], start=False, stop=True)
        # dterm[(cl,t), cc] = SMr[t] k2r[c] + SMi[t] k2i[c] + bias_p[c]
        nc.tensor.matmul(pk[:, 2, 0:NCC], lhsT=d_a,
                         rhs=kmask[:, 2, :], start=True, stop=False)
        nc.tensor.matmul(pk[:, 2, 0:NCC], lhsT=d_b,
                         rhs=kmask[:, 3, :], start=False, stop=False)
        nc.tensor.matmul(pk[:, 2, 0:NCC], lhsT=selp,
                         rhs=bpm, start=False, stop=True)
        nc.scalar.activation(out=zvec_a, in_=pk[0:126, 0, 0:NCC], func=AF.Copy)
        nc.scalar.activation(out=zvec_b, in_=pk[0:18, 1, 0:NCC], func=AF.Copy)
        nc.scalar.activation(out=dterm, in_=pk[:, 2, 0:NCC], func=AF.Copy)

        # =========== Phase B5: scale, irfft, dterm =========================
        with tc.tile_pool(name="zld", bufs=2) as zld, \
                tc.tile_pool(name="finp", bufs=3) as fin_p:
            for cc in range(NCC):
                za = zld.tile([126, NCHUNK, CHW], F16, tag="za", name=f"za{cc}")
                nc.sync.dma_start(
                    out=za.rearrange("p c x -> p (c x)"),
                    in_=z_dram[cc * CL:cc * CL + 7].rearrange(
                        "o q x -> (o q) x"))
                zb = zld.tile([18, NCHUNK, CHW], F16, tag="zb", name=f"zb{cc}")
                nc.sync.dma_start(
                    out=zb.rearrange("p c x -> p (c x)"),
                    in_=z_dram[cc * CL + 7])
                zsa = zld.tile([126, NCHUNK, CHW], F16, tag="zsa",
                               name=f"zsa{cc}")
                nc.vector.tensor_scalar_mul(zsa, za, zvec_a[:, cc:cc + 1])
                zsb = zld.tile([18, NCHUNK, CHW], F16, tag="zsb",
                               name=f"zsb{cc}")
                nc.vector.tensor_scalar_mul(zsb, zb, zvec_b[:, cc:cc + 1])
                pf = ps.tile([128, NCHUNK, 512], F32, tag="ps", name=f"pfin{cc}")
                for j in range(NCHUNK):
                    nc.tensor.matmul(pf[:, j, 0:CHW], lhsT=ira,
                                     rhs=zsa[:, j, :], start=True, stop=False)
                    nc.tensor.matmul(pf[:, j, 0:CHW], lhsT=irb,
                                     rhs=zsb[:, j, :], start=False, stop=True)
                fin = fin_p.tile([128, NCHUNK, CHW], F16, tag="fin",
                                 name=f"fin{cc}")
                nc.vector.tensor_scalar_add(fin, pf[:, :, 0:CHW],
                                            dterm[:, cc:cc + 1])
                nc.sync.dma_start(
                    out=out_d[cc].rearrange("p h w -> p (h w)"),
                    in_=fin.rearrange("p c x -> p (c x)"))
        ps_pool.__exit__(None, None, None)

    return nc


def _get_program():
    if "nc" not in _CACHE:
        _CACHE["nc"] = _build_program()
    return _CACHE["nc"]


# ------------------------------------------------------------------ host --
def _make_in_maps(inputs):
    x = np.asarray(inputs["x"], np.float32)
    consts = _build_consts(
        np.asarray(inputs["weights_cor"], np.float32),
        np.asarray(inputs["Wg"], np.float32),
        np.asarray(inputs["w1r"], np.float32),
        np.asarray(inputs["w1i"], np.float32),
        np.asarray(inputs["Wlr"], np.float32),
        np.asarray(inputs["Wli"], np.float32),
        np.asarray(inputs["gamma_r"], np.float32),
        np.asarray(inputs["beta_r"], np.float32),
        np.asarray(inputs["gamma_i"], np.float32),
        np.asarray(inputs["beta_i"], np.float32),
        np.asarray(inputs["alpha1"], np.float32),
        np.asarray(inputs["bias_p"], np.float32),
    )
    # padded full tensor (zero pad h by 1, w by 2 on each side)
    xp = np.pad(x, ((0, 0), (0, 0), (0, 0), (1, 1), (2, 2))).astype(H16)

    in_maps = []
    for core in range(8):
        b, hf = core // 2, core % 2
        h0 = hf * Hh
        sh = xp[b, :, :, h0:h0 + HP, :]          # [C, T, HP, WP]
        # partition = (t*8 + c%8), free = (c//8, HP, WP)
        sh = sh.reshape(NCC, CL, T, HP, WP).transpose(2, 1, 0, 3, 4)
        d = {"x_p": np.ascontiguousarray(sh.reshape(128, NCC, HP, WP))}
        d.update(consts)
        in_maps.append(d)
    return in_maps


def kernel(**inputs):
    in_maps = _make_in_maps(inputs)
    nc = _get_program()
    res = run_bass_kernel_spmd(nc, in_maps, core_ids=list(range(8)))
    x = np.asarray(inputs["x"], np.float32)
    out = np.empty((B, C, T, H, W), np.float32)
    for core in range(8):
        b, hf = core // 2, core % 2
        # core result: [NCC, 128=(cl*16+t), Hh, W] -> [C, T, Hh, W]
        r = res.results[core]["out"].astype(np.float32).reshape(C, T, Hh, W)
        sl = np.s_[b, :, :, hf * Hh:(hf + 1) * Hh, :]
        out[sl] = r + x[sl]
    return out
